# revision 12
# baseline (speedup 1.0000x reference)
"""LocalGaussianBlur3D on 8 Trainium2 NeuronCores.

The reference blurs the whole [1,256,256,256] volume with a 9x9x9 Gaussian
but only keeps the blurred values inside the union of (2R+1)^3 boxes around
<=6 points; everywhere else the output equals the input.  The separable
blur of the <=6 17^3 input patches is the ONLY real compute, so the device
program is exactly that:

  * DMA in the zero-padded 17^3 patches around each point (one small fp16
    transfer); the banded z-conv weight matrix is a compile-time constant
    DMA'd concurrently on a second queue,
  * z-pass as two banded matmuls on the tensor engine (one per y-halving
    group, so the downstream free dims halve: 54->108 partitions),
  * y- and x-pass on the vector engine, restructured around the kernel's
    symmetry: 4 pair-adds (tensor_tensor, 2x mode in fp16) + a 5-term
    scaled accumulation chain per pass, instead of 9 serial 1x FMAs,
  * DMA out the blurred boxes (fp16; host casts back to f32).

Sharding: the patch work is replicated SPMD on all 8 cores (it is latency-
bound, not throughput-bound; splitting the 6 boxes across cores does not
shorten the serial chain).  Host side slices/zero-pads the patches
(sharding) and unshards by overlaying the <=6 blurred 9^3 boxes onto the
unchanged volume.

The device program is geometry-independent: box positions only affect host
slicing, so the same compiled NEFF handles any points.
"""

import numpy as np

R = 4
SIGMA = 1.2
K = 2 * R + 1        # 9 taps
PATCH = 4 * R + 1    # 17: input patch edge for a 9^3 output box
XP = PATCH + 1       # 18: x-extent padded for fp16 slice alignment
YW = PATCH - R       # 13: y-window rows needed per y-halving group
YO = K - R           # 5: y outputs per group (group 1 recomputes yo=4)
PB = 64              # partition base of group 1 (PSUM matmul base rule)
D = H = W = 256
NCORES = 8


def _gauss1d():
    x = np.arange(K, dtype=np.float32) - np.float32((K - 1) / 2)
    g = np.exp(-(x * x) / np.float32(2.0 * SIGMA * SIGMA)).astype(np.float32)
    return (g / np.maximum(g.sum(dtype=np.float32), np.float32(1e-12))).astype(
        np.float32
    )


def _wz_matrix(n_boxes):
    """Banded [n_boxes*17, n_boxes*9] matrix: z-conv as lhsT of one matmul."""
    g = _gauss1d()
    wz = np.zeros((n_boxes * PATCH, n_boxes * K), np.float32)
    for b in range(n_boxes):
        for zo in range(K):
            for dz in range(K):
                wz[b * PATCH + zo + dz, b * K + zo] = g[dz]
    return wz


def build_bass(n_boxes):
    from concourse import bass, mybir

    f16 = mybir.dt.float16
    mult, add = mybir.AluOpType.mult, mybir.AluOpType.add
    nc = bass.Bass()
    P = n_boxes * PATCH          # matmul contraction partitions (<=128)
    PZ = n_boxes * K             # partitions per y-halving group
    PG = PB + PZ                 # total partitions after the z-pass
    assert PZ <= PB and PG <= 128
    YX = PATCH * XP              # 17*18 free elems per input partition
    aux = nc.dram_tensor("aux", [P, YX], f16, kind="ExternalInput")
    # group 0's lhsT is padded to 64 output columns with zeros so the
    # psum gap rows (PZ..PB) are written with zeros, not left as garbage
    wz_np = _wz_matrix(n_boxes).astype(np.float16)
    wz_pad = np.zeros((P, PB + PZ), np.float16)
    wz_pad[:, :PZ] = wz_np
    wz_pad[:, PB:] = wz_np
    wz_const = nc.inline_tensor(wz_pad, name="wz_const")
    pout = nc.dram_tensor("pout", [PG, YO * K], f16, kind="ExternalOutput")

    g = _gauss1d()

    with (
        nc.sbuf_tensor([P, YX], f16) as a_t,            # input patches
        nc.sbuf_tensor([P, PB + PZ], f16) as wz2,       # z-conv weights
        nc.psum_tensor([PG, YW * XP], mybir.dt.float32) as zp,
        nc.sbuf_tensor([PG, YW * XP], f16) as s1,       # z-blurred patches
        nc.sbuf_tensor([PG, 4 * YO * XP], f16) as yp,   # y pair sums
        nc.sbuf_tensor([PG, YO * XP], f16) as yt0,
        nc.sbuf_tensor([PG, YO * XP], f16) as yt1,
        nc.sbuf_tensor([PG, 4 * YO * K], f16) as xq,    # x pair sums
        nc.sbuf_tensor([PG, YO * K], f16) as xt0,
        nc.sbuf_tensor([PG, YO * K], f16) as xt1,
        nc.semaphore("in_sem") as in_sem,
        nc.semaphore("wz_sem") as wz_sem,
        nc.semaphore("dve_sem") as dve_sem,
        nc.semaphore("pe_sem") as pe_sem,
        nc.semaphore("st_sem") as st_sem,
        nc.Block() as block,
    ):
        a3 = a_t[:].rearrange("p (y x) -> p y x", y=PATCH)
        z3 = zp[:].rearrange("p (y x) -> p y x", y=YW)
        s3 = s1[:].rearrange("p (y x) -> p y x", y=YW)
        yp4 = yp[:].rearrange("p (d y x) -> p d y x", d=4, y=YO)
        yt = [yt0[:].rearrange("p (y x) -> p y x", y=YO),
              yt1[:].rearrange("p (y x) -> p y x", y=YO)]
        xq4 = xq[:].rearrange("p (d y x) -> p d y x", d=4, y=YO)
        xt = [xt0[:].rearrange("p (y x) -> p y x", y=YO),
              xt1[:].rearrange("p (y x) -> p y x", y=YO)]

        @block.sync
        def _(s):
            s.dma_start(out=a_t[:], in_=aux[:]).then_inc(in_sem, 16)
            s.wait_ge(st_sem, 16)

        @block.tensor
        def _(t):
            t.wait_ge(wz_sem, 16)
            t.wait_ge(in_sem, 16)
            # z-pass, one banded matmul per y-halving group
            t.matmul(out=z3[:PB], lhsT=wz2[:, :PB],
                     rhs=a3[:, 0:YW, :],
                     start=True, stop=True).then_inc(pe_sem, 1)
            t.matmul(out=z3[PB:], lhsT=wz2[:, PB:],
                     rhs=a3[:, PATCH - YW : PATCH, :],
                     start=True, stop=True).then_inc(pe_sem, 1)

        # the DVE pipeline doesn't interlock consecutive instructions, so
        # every dependent op gets a sem handoff (waits on producers' counts)
        @block.vector
        def _(v):
            v.wait_ge(pe_sem, 2)
            # 1: bounce the z-pass result out of PSUM (cast to fp16)
            v.tensor_copy(s1[:], zp[:]).then_inc(dve_sem, 1)
            # y pass, symmetric taps: out = sum_d g[d]*(w_d + w_{8-d})
            #   + g[4]*w_4, with w_dy = s3[:, dy:dy+5, :]
            for d in range(4):  # 2..5: pair sums
                v.wait_ge(dve_sem, 1)
                v.tensor_tensor(
                    out=yp4[:, d], in0=s3[:, d : d + YO, :],
                    in1=s3[:, R * 2 - d : R * 2 - d + YO, :], op=add,
                ).then_inc(dve_sem, 1)
            # 6: t = g0*p0;  7-9: t += g_d*p_d;  10: out = g4*w4 + t
            v.wait_ge(dve_sem, 2)
            v.tensor_scalar_mul(yt[0], yp4[:, 0], float(g[0])).then_inc(
                dve_sem, 1)
            for d in range(1, 4):
                v.wait_ge(dve_sem, 5 + d)
                v.scalar_tensor_tensor(
                    out=yt[d % 2], in0=yp4[:, d], scalar=float(g[d]),
                    in1=yt[1 - d % 2], op0=mult, op1=add).then_inc(dve_sem, 1)
            v.wait_ge(dve_sem, 9)
            v.scalar_tensor_tensor(
                out=yt[0], in0=s3[:, R : R + YO, :], scalar=float(g[R]),
                in1=yt[1], op0=mult, op1=add).then_inc(dve_sem, 1)
            by = yt[0]
            # x pass, same structure; w_dx = by[:, :, dx:dx+9]
            for d in range(4):  # 11..14: pair sums
                v.wait_ge(dve_sem, 10)
                v.tensor_tensor(
                    out=xq4[:, d], in0=by[:, :, d : d + K],
                    in1=by[:, :, R * 2 - d : R * 2 - d + K], op=add,
                ).then_inc(dve_sem, 1)
            v.wait_ge(dve_sem, 11)
            v.tensor_scalar_mul(xt[0], xq4[:, 0], float(g[0])).then_inc(
                dve_sem, 1)
            for d in range(1, 4):
                v.wait_ge(dve_sem, 14 + d)
                v.scalar_tensor_tensor(
                    out=xt[d % 2], in0=xq4[:, d], scalar=float(g[d]),
                    in1=xt[1 - d % 2], op0=mult, op1=add).then_inc(dve_sem, 1)
            v.wait_ge(dve_sem, 18)
            v.scalar_tensor_tensor(
                out=xt[0], in0=by[:, :, R : R + K], scalar=float(g[R]),
                in1=xt[1], op0=mult, op1=add).then_inc(dve_sem, 1)

        n_chain = 19

        @block.scalar
        def _(sc):
            sc.dma_start(out=wz2[:], in_=wz_const[:]).then_inc(wz_sem, 16)
            sc.wait_ge(dve_sem, n_chain)
            sc.dma_start(out=pout[:], in_=xt0[:]).then_inc(st_sem, 16)

    return nc


def input_specs(n_boxes):
    """name -> shape of the device ExternalInputs (for sim tooling)."""
    return {"aux": (n_boxes * PATCH, PATCH * XP)}


_NC_CACHE = {}


def _boxes(points):
    """Per point: clipped output box and where the patch maps into it."""
    out = []
    for pz, py, px in points:
        lo = [max(0, c - R) for c in (pz, py, px)]
        hi = [min(D, c + R + 1) for c in (pz, py, px)]
        off = [l - (c - R) for l, c in zip(lo, (pz, py, px))]
        out.append((lo, hi, off))
    return out


def kernel(volume, points):
    return _run(volume, points)[0]


def _run(volume, points, trace=False):
    volume = np.ascontiguousarray(np.asarray(volume, dtype=np.float32))
    points = [tuple(int(c) for c in p) for p in np.asarray(points)]
    vol = volume[0]
    nb = len(points)

    # zero-padded 17^3 input patches (zero padding == conv's border
    # behavior), x-extent padded to 18 for fp16 slice alignment
    pin = np.zeros((nb, PATCH, PATCH, XP), np.float16)
    for i, (pz, py, px) in enumerate(points):
        sl_src, sl_dst = [], []
        for c in (pz, py, px):
            s0, s1 = max(0, c - 2 * R), min(D, c + 2 * R + 1)
            sl_src.append(slice(s0, s1))
            sl_dst.append(slice(s0 - (c - 2 * R), s1 - (c - 2 * R)))
        pin[i][tuple(sl_dst)] = vol[tuple(sl_src)]

    if nb not in _NC_CACHE:
        _NC_CACHE[nb] = build_bass(nb)
    nc = _NC_CACHE[nb]

    from concourse.bass_utils import run_bass_kernel_spmd

    aux = pin.reshape(nb * PATCH, PATCH * XP)
    in_maps = [{"aux": aux} for _ in range(NCORES)]
    res = run_bass_kernel_spmd(
        nc, in_maps, core_ids=list(range(NCORES)), trace=trace
    )

    # reassemble the blurred boxes from the two y-halving groups:
    # group 0 holds yo 0..4, group 1 holds yo 4..8 (row 0 is a duplicate)
    praw = res.results[0]["pout"].astype(np.float32)
    g0 = praw[: nb * K].reshape(nb, K, YO, K)     # (b, zo, yo_l, xo)
    g1 = praw[PB : PB + nb * K].reshape(nb, K, YO, K)
    pout = np.empty((nb, K, K, K), np.float32)    # (b, zo, yo, xo)
    pout[:, :, :YO] = g0
    pout[:, :, YO:] = g1[:, :, 1:]

    # unshard: output == input everywhere except the <=6 blurred boxes
    out = vol.copy()
    for i, (lo, hi, off) in enumerate(_boxes(points)):
        out[lo[0] : hi[0], lo[1] : hi[1], lo[2] : hi[2]] = pout[i][
            off[0] : off[0] + hi[0] - lo[0],
            off[1] : off[1] + hi[1] - lo[1],
            off[2] : off[2] + hi[2] - lo[2],
        ]
    return out[None], res
